# revision 48
# baseline (speedup 1.0000x reference)
"""Trainium2 Bass kernel for nn_AttentionLateralOp (lateral self-attention).

Reference computation (B=4, C=512, H=W=64, N=H*W=4096, CQ=C//8=64):
    f  = Wq @ x_t            # [B, CQ, N]   query from target
    g  = Wk @ x_o            # [B, CQ, N]   key from origin
    hh = Wv @ x_o            # [B, C,  N]   value from origin
    scores[m, n] = sum_q f[q, m] * g[q, n]          # [B, N, N]
    beta = softmax(scores, axis=m)
    out[c, n] = gamma * sum_m hh[c, m] * beta[m, n] + x_t[c, n]

Sharding: 8 cores = (batch b in 0..3) x (half of the n axis).  Each core
holds full f/hh for its batch (softmax is over the full m axis) and a
2048-wide slice of g / x_t / output.  No collectives needed.

Per-core algorithm (v2: fp8 DoubleRow for the O(N^2 C) work):
    - f  = WqT^T @ x_t  (bf16, K=C tiled by 128)   -> SBUF bf16 [128, 8, 512]
    - g  = WkT^T @ x_o_slice (bf16)                -> SBUF bf16 [128, 4, 512]
      (f/g stored 128-tall with zeroed upper partitions)
    - hh_T[m, c] = x_o^T @ Wv^T in fp8e4m3 DoubleRow (2 c-tile pairs per
      instr), cast to e4m3                         -> SBUF [128, 32, 512]
    - per 512-wide n-chunk (two-pass exact softmax, m-tiles in pairs):
        pass A: scores_psum[128, 2, 512] = f_pair^T @ g_chunk (bf16)
                E = exp(scores - 40) -> bf16 SBUF  (one act per pair)
                s_psum += ones^T @ E               (PE column sums)
                rec = 1/s;  beta = E * rec -> fp8e4m3 (DVE/Pool split)
        pass B: o_psum[c_tile] += hh_pair^T @ beta_pair  (fp8 DoubleRow,
                K=256 per instr), 2 c-tiles per PSUM tile, 2 half-passes
        out = gamma * o + x_t   (scalar_tensor_tensor, bf16 out)
      Pass A of chunk k+1 is emitted before pass B of chunk k so the Act
      engine (exp floor ~17us/chunk) never starves behind PE o-matmuls.
"""

import os
import threading

import numpy as np
import ml_dtypes

import concourse.bass as bass
import concourse.tile as tile
from concourse import bacc, mybir
from concourse.bass_utils import run_bass_kernel_spmd

B = 4
C = 512
HW = 64
N = HW * HW          # 4096
CQ = 64              # query/key channels
P = 128              # partitions
CT = C // P          # 4  c-tiles
MT = N // P          # 32 m-tiles
MP = MT // 2         # 16 m-tile pairs
NCORES = 8
NSL = N // (NCORES // B)      # 2048: n-slice per core
NCH = 512                     # n-chunk
NCHUNKS = NSL // NCH          # 4

F32 = mybir.dt.float32
BF16 = mybir.dt.bfloat16
E4 = mybir.dt.float8e4
DR = mybir.MatmulPerfMode.DoubleRow
MULT = mybir.AluOpType.mult
ADD = mybir.AluOpType.add

# beta multiplies routed to the Pool engine (rest on DVE)
POOL_MULS = (0, 2, 4, 5, 7, 8, 10, 11, 13, 14)
# E-pair pre-sums on Pool (one ones-matmul instead of two for these pairs)
POOL_SUMS = (0, 1, 2, 3, 5, 6, 7, 9, 10, 11, 13, 14)


def _build_bass(reps=1):
    nc = bacc.Bacc(trn_type="TRN2")

    xt_full = nc.dram_tensor("xt_full", [C, N], BF16, kind="ExternalInput")
    xo8_full = nc.dram_tensor("xo8_full", [C, N], E4, kind="ExternalInput")
    xo_sl = nc.dram_tensor("xo_sl", [C, NSL], BF16, kind="ExternalInput")
    xt_sl = nc.dram_tensor("xt_sl", [C, NSL], BF16, kind="ExternalInput")
    wq_t = nc.dram_tensor("wq_t", [C, CQ], BF16, kind="ExternalInput")
    wk_t = nc.dram_tensor("wk_t", [C, CQ], BF16, kind="ExternalInput")
    wv_t = nc.dram_tensor("wv_t", [C, C], E4, kind="ExternalInput")
    gamma = nc.dram_tensor("gamma", [1, 1], F32, kind="ExternalInput")
    out = nc.dram_tensor("out", [C, NSL], BF16, kind="ExternalOutput")

    with tile.TileContext(nc) as tc:
        with (
            tc.tile_pool(name="const", bufs=1) as const,
            tc.tile_pool(name="epool", bufs=24) as epool,
            tc.tile_pool(name="bpool", bufs=22) as bpool,
            tc.tile_pool(name="work", bufs=4) as work,
            tc.tile_pool(name="ps_sc", bufs=2, space="PSUM") as ps_sc,
            tc.tile_pool(name="ps_s", bufs=1, space="PSUM") as ps_s,
            tc.tile_pool(name="ps_o", bufs=1, space="PSUM") as ps_o,
        ):
            # ---- weights / constants (DMA order = consumption order:
            # wq/wk/xo_sl feed g, then xt chunks feed f, then hh inputs,
            # xt_sl last -- it is only read by the first pass_b finals) ----
            wq_sb = const.tile([P, CT, CQ], BF16)
            nc.sync.dma_start(wq_sb, wq_t.rearrange("(ct p) q -> p ct q", p=P))
            wk_sb = const.tile([P, CT, CQ], BF16)
            nc.sync.dma_start(wk_sb, wk_t.rearrange("(ct p) q -> p ct q", p=P))
            xosl_sb = const.tile([P, CT, NSL], BF16)
            xosl_r = xo_sl.rearrange("(ct p) n -> p ct n", p=P)
            nc.sync.dma_start(xosl_sb[:, :, 0:NSL // 2], xosl_r[:, :, 0:NSL // 2])
            xt_sb = const.tile([P, CT, N], BF16)
            xt_r = xt_full.rearrange("(ct p) n -> p ct n", p=P)
            for j in range(4):
                sl = slice(j * (N // 4), (j + 1) * (N // 4))
                nc.sync.dma_start(xt_sb[:, :, sl], xt_r[:, :, sl])
            wv_sb = const.tile([P, CT, C], E4)
            nc.sync.dma_start(wv_sb, wv_t.rearrange("(ct p) c -> p ct c", p=P))
            xo8_sb = const.tile([P, CT, N], E4)
            nc.sync.dma_start(xo8_sb, xo8_full.rearrange("(ct p) n -> p ct n", p=P))
            nc.sync.dma_start(xosl_sb[:, :, NSL // 2:], xosl_r[:, :, NSL // 2:])
            xtsl_sb = const.tile([P, CT, NSL], BF16)
            nc.sync.dma_start(xtsl_sb, xt_sl.rearrange("(ct p) n -> p ct n", p=P))
            gamma_sb = const.tile([P, 1], F32)
            nc.gpsimd.dma_start(out=gamma_sb, in_=gamma[:, :].to_broadcast([P, 1]))
            ones_sb = const.tile([P, P], BF16)
            nc.vector.memset(ones_sb, 1.0)
            expbias_sb = const.tile([P, 1], F32)
            nc.vector.memset(expbias_sb, -40.0)
            # warm the Act engine's exp table during the input-DMA window so
            # the first real exp doesn't pay the table load
            actwarm_sb = const.tile([P, 1], F32)
            nc.scalar.activation(
                actwarm_sb, expbias_sb, mybir.ActivationFunctionType.Exp,
                bias=expbias_sb, scale=1.0,
            )

            f_sb = const.tile([P, N // NCH, NCH], BF16)      # [128, 8, 512]
            nc.vector.memset(f_sb[CQ:P, :, :], 0.0)
            g_sb = const.tile([P, NCHUNKS, NCH], BF16)       # [128, 4, 512]
            nc.vector.memset(g_sb[CQ:P, :, :], 0.0)
            hh_sb = const.tile([P, MT, C], E4)               # [128, 32, 512]

            for _rep in range(reps):
                # ---- attention chunks, software-pipelined ----
                state = {}

                def emit_g_pair(j):
                    # g chunks (2j, 2j+1) = Wk @ x_o_slice cols
                    ps = ps_sc.tile([P, 2, NCH], F32, tag="sc", name="g_ps")
                    for i in range(2):
                        gc = 2 * j + i
                        for ci in range(CT):
                            nc.tensor.matmul(
                                ps[:CQ, i, :],
                                wk_sb[:, ci, :],
                                xosl_sb[:, ci, gc * NCH:(gc + 1) * NCH],
                                start=(ci == 0),
                                stop=(ci == CT - 1),
                            )
                    nc.vector.tensor_copy(
                        out=g_sb[:CQ, 2 * j:2 * j + 2, :], in_=ps[:CQ, :, :]
                    )

                def emit_f_pair(j):
                    # f chunks (2j, 2j+1) = Wq @ x_t cols
                    ps = ps_sc.tile([P, 2, NCH], F32, tag="sc", name="f_ps")
                    for i in range(2):
                        mc = 2 * j + i
                        for ci in range(CT):
                            nc.tensor.matmul(
                                ps[:CQ, i, :],
                                wq_sb[:, ci, :],
                                xt_sb[:, ci, mc * NCH:(mc + 1) * NCH],
                                start=(ci == 0),
                                stop=(ci == CT - 1),
                            )
                    nc.vector.tensor_copy(
                        out=f_sb[:CQ, 2 * j:2 * j + 2, :], in_=ps[:CQ, :, :]
                    )

                def emit_hh_pair(j):
                    # hh_T[m, c] for m-tiles (2j, 2j+1), fp8 DoubleRow
                    ps = ps_sc.tile([P, 2, C], F32, tag="sc", name="hh_ps")
                    for i in range(2):
                        mi = 2 * j + i
                        for cp in range(CT // 2):
                            nc.tensor.matmul(
                                ps[:, i, :],
                                xo8_sb[:, 2 * cp:2 * cp + 2,
                                       mi * P:(mi + 1) * P],
                                wv_sb[:, 2 * cp:2 * cp + 2, :],
                                start=(cp == 0),
                                stop=(cp == CT // 2 - 1),
                                perf_mode=DR,
                            )
                    nc.vector.tensor_copy(
                        out=hh_sb[:, 2 * j:2 * j + 2, :], in_=ps
                    )

                # Production of hh / remaining f / second-half g rides inside
                # the chunk pair loops, placed where the feeding DMA has
                # landed and each is emitted before its first consumer.
                hh_sched = {(0, p): p - 6 for p in range(6, 16)}
                hh_sched.update({(1, p): p + 10 for p in range(6)})
                f_sched = {(0, 0): 1, (0, 1): 2, (0, 3): 3}
                g_sched = {(0, 8): 1}

                def emit_odr(betas, half, q, o_ps):
                    # o-matmul dr-pair q (m-tiles 2q,2q+1) for c-half `half`
                    for cj in range(2):
                        ci = 2 * half + cj
                        nc.tensor.matmul(
                            o_ps[:, cj, :],
                            hh_sb[:, 2 * q:2 * q + 2, ci * P:(ci + 1) * P],
                            betas[q],
                            start=(q == 0),
                            stop=(q == MP - 1),
                            perf_mode=DR,
                        )

                def emit_finals(ch_prev, half, o_ps):
                    nsl = slice(ch_prev * NCH, (ch_prev + 1) * NCH)
                    for cj in range(2):
                        ci = 2 * half + cj
                        os_ = work.tile([P, NCH], BF16, tag="os", name="os")
                        nc.vector.scalar_tensor_tensor(
                            out=os_,
                            in0=o_ps[:, cj, :],
                            scalar=gamma_sb,
                            in1=xtsl_sb[:, ci, nsl],
                            op0=MULT,
                            op1=ADD,
                        )
                        nc.sync.dma_start(out[ci * P:(ci + 1) * P, nsl], os_)

                def pass_a(ch):
                    s_ps = ps_s.tile([P, NCH], F32, tag="s", name="s_ps")
                    ets = []
                    pend = None   # s-matmul for pair p emitted at pair p+1
                    first_s = [True]
                    prev = ch - 1 if ch >= 1 else None
                    pbetas = state.pop(prev) if prev is not None else None
                    o_ps = [None]

                    def emit_s(item, last):
                        kind, t = item
                        if kind == "es":
                            nc.tensor.matmul(
                                s_ps, ones_sb, t,
                                start=first_s[0], stop=last,
                            )
                        else:
                            nc.tensor.matmul(
                                s_ps, ones_sb, t[:, 0, :],
                                start=first_s[0], stop=False,
                            )
                            nc.tensor.matmul(
                                s_ps, ones_sb, t[:, 1, :],
                                start=False, stop=last,
                            )
                        first_s[0] = False

                    for p in range(MP):
                        sc = ps_sc.tile([P, 2, NCH], F32, tag="sc", name="sc_ps")
                        for i in range(2):
                            mi = 2 * p + i
                            nc.tensor.matmul(
                                sc[:, i, :],
                                f_sb[:, mi // 4, (mi % 4) * P:(mi % 4 + 1) * P],
                                g_sb[:, ch, :],
                                start=True,
                                stop=True,
                            )
                        et = epool.tile([P, 2, NCH], BF16, tag="et", name="et")
                        nc.scalar.activation(
                            et, sc, mybir.ActivationFunctionType.Exp,
                            bias=expbias_sb, scale=1.0,
                        )
                        if (ch, p) in f_sched:
                            emit_f_pair(f_sched[(ch, p)])
                        if (ch, p) in g_sched:
                            emit_g_pair(g_sched[(ch, p)])
                        if (ch, p) in hh_sched:
                            emit_hh_pair(hh_sched[(ch, p)])
                        if p in POOL_SUMS:
                            es = work.tile([P, NCH], BF16, tag="es",
                                           bufs=12, name="es")
                            nc.gpsimd.tensor_add(
                                out=es, in0=et[:, 0, :], in1=et[:, 1, :]
                            )
                            item = ("es", es)
                        else:
                            item = ("et", et)
                        if pend is not None:
                            emit_s(pend, last=False)
                        pend = item
                        ets.append(et)
                        # previous chunk's o-matmuls ride pair-by-pair, two
                        # pairs late so beta production stays ahead of the
                        # in-order PE queue: c-half 0 over pairs 2-9, c-half
                        # 1 over pairs 10-15 plus a post-loop tail
                        if pbetas is not None:
                            if 2 <= p < 10:
                                if p == 2:
                                    o_ps[0] = ps_o.tile([P, 2, NCH], F32,
                                                        tag="o", name="o_ps")
                                emit_odr(pbetas, 0, 2 * (p - 2), o_ps[0])
                                emit_odr(pbetas, 0, 2 * (p - 2) + 1, o_ps[0])
                            elif p >= 10:
                                if p == 10:
                                    emit_finals(prev, 0, o_ps[0])
                                    o_ps[0] = ps_o.tile([P, 2, NCH], F32,
                                                        tag="o", name="o_ps")
                                emit_odr(pbetas, 1, 2 * (p - 10), o_ps[0])
                                emit_odr(pbetas, 1, 2 * (p - 10) + 1, o_ps[0])
                    emit_s(pend, last=True)
                    if pbetas is not None:
                        for dq in range(12, MP):
                            emit_odr(pbetas, 1, dq, o_ps[0])
                        emit_finals(prev, 1, o_ps[0])
                    rec = work.tile([P, 2, NCH], F32, tag="rec", bufs=2, name="rec")
                    nc.vector.reciprocal_approx_fast(out=rec[:, 0, :], in_=s_ps)
                    nc.vector.tensor_copy(out=rec[:, 1, :], in_=rec[:, 0, :])
                    betas = []
                    for p in range(MP):
                        bt = bpool.tile([P, 2, NCH], E4, tag="bt", name="bt")
                        eng = nc.gpsimd if p in POOL_MULS else nc.vector
                        eng.tensor_mul(out=bt, in0=ets[p], in1=rec)
                        betas.append(bt)
                    state[ch] = betas

                def pass_b(ch):
                    nsl = slice(ch * NCH, (ch + 1) * NCH)
                    betas = state.pop(ch)
                    for half in range(2):
                        o_ps = ps_o.tile([P, 2, NCH], F32, tag="o", name="o_ps")
                        for p in range(MP):
                            for cj in range(2):
                                ci = 2 * half + cj
                                nc.tensor.matmul(
                                    o_ps[:, cj, :],
                                    hh_sb[:, 2 * p:2 * p + 2,
                                          ci * P:(ci + 1) * P],
                                    betas[p],
                                    start=(p == 0),
                                    stop=(p == MP - 1),
                                    perf_mode=DR,
                                )
                        for cj in range(2):
                            ci = 2 * half + cj
                            os_ = work.tile([P, NCH], BF16, tag="os", name="os")
                            nc.vector.scalar_tensor_tensor(
                                out=os_,
                                in0=o_ps[:, cj, :],
                                scalar=gamma_sb,
                                in1=xtsl_sb[:, ci, nsl],
                                op0=MULT,
                                op1=ADD,
                            )
                            nc.sync.dma_start(out[ci * P:(ci + 1) * P, nsl], os_)

                emit_g_pair(0)
                emit_f_pair(0)
                for ch in range(NCHUNKS):
                    pass_a(ch)
                pass_b(NCHUNKS - 1)
    nc.compile()
    return nc


_lock = threading.Lock()
_cached_nc = None


def _get_nc():
    global _cached_nc
    with _lock:
        if _cached_nc is None:
            _cached_nc = _build_bass()
        return _cached_nc


def make_in_maps(origin_out, target_in, Wq, Wk, Wv, gamma):
    bf = ml_dtypes.bfloat16
    e4 = ml_dtypes.float8_e4m3
    x_o = np.ascontiguousarray(np.asarray(origin_out, dtype=np.float32).reshape(B, C, N))
    x_t = np.ascontiguousarray(np.asarray(target_in, dtype=np.float32).reshape(B, C, N))
    x_t_bf = x_t.astype(bf)
    x_o_bf = x_o.astype(bf)
    x_o_8 = x_o.astype(e4)
    wq_t = np.ascontiguousarray(np.asarray(Wq, dtype=np.float32).T).astype(bf)
    wk_t = np.ascontiguousarray(np.asarray(Wk, dtype=np.float32).T).astype(bf)
    wv_t = np.ascontiguousarray(np.asarray(Wv, dtype=np.float32).T).astype(e4)
    gam = np.asarray(gamma, dtype=np.float32).reshape(1, 1)

    in_maps = []
    for core in range(NCORES):
        b = core // (NCORES // B)
        h = core % (NCORES // B)
        sl = slice(h * NSL, (h + 1) * NSL)
        in_maps.append(
            {
                "xt_full": x_t_bf[b],
                "xo8_full": x_o_8[b],
                "xo_sl": np.ascontiguousarray(x_o_bf[b][:, sl]),
                "xt_sl": np.ascontiguousarray(x_t_bf[b][:, sl]),
                "wq_t": wq_t,
                "wk_t": wk_t,
                "wv_t": wv_t,
                "gamma": gam,
            }
        )
    return in_maps


def assemble_output(results):
    out = np.empty((B, C, N), dtype=np.float32)
    for core in range(NCORES):
        b = core // (NCORES // B)
        h = core % (NCORES // B)
        sl = slice(h * NSL, (h + 1) * NSL)
        out[b][:, sl] = results[core]["out"].astype(np.float32)
    return out.reshape(B, C, HW, HW)


def kernel(origin_out, target_in, Wq, Wk, Wv, gamma):
    nc = _get_nc()
    in_maps = make_in_maps(origin_out, target_in, Wq, Wk, Wv, gamma)
    res = run_bass_kernel_spmd(nc, in_maps, core_ids=list(range(NCORES)))
    return assemble_output(res.results)


if __name__ == "__main__":
    rng = np.random.default_rng(0)
    inputs = {
        "origin_out": rng.standard_normal((B, C, HW, HW), dtype=np.float32),
        "target_in": rng.standard_normal((B, C, HW, HW), dtype=np.float32),
        "Wq": (rng.standard_normal((CQ, C)) / np.sqrt(C)).astype(np.float32),
        "Wk": (rng.standard_normal((CQ, C)) / np.sqrt(C)).astype(np.float32),
        "Wv": (rng.standard_normal((C, C)) / np.sqrt(C)).astype(np.float32),
        "gamma": np.ones((1,), dtype=np.float32),
    }
    out = kernel(**inputs)
    print("kernel output", out.shape, out.dtype, float(np.abs(out).mean()))
